# revision 102
# baseline (speedup 1.0000x reference)
"""Trainium2 Bass kernel for a post-norm transformer encoder layer.

Contract: kernel(**inputs) takes the FULL fp32 inputs (as produced by the
problem's setup_inputs) and returns the FULL [2, 2048, 512] fp32 output.

Sharding (8 cores, no collectives): core c owns 512 query tokens of batch
c // 4 (slice (c % 4) * 512). Each core recomputes the K/V projections for
its whole batch (2048 tokens) and runs attention + FFN for its 512 queries.

Fast path: every GEMM runs as fp8e4 DoubleRow matmuls (2 contraction tiles
per instruction at 0.5 cycles/row). Weights are host-scaled by 64 into the
fp8 normal range; every psum consumer applies the inverse power-of-two
scale for free inside the op it already needed. Softmax exp is split
between the ACT engine (Exp) and gpsimd (pow with an f32 e^(1/128) base,
fed by a DVE psum->sbuf copy that also releases the score bank; Pool
slots sit at the head-pair boundary slots so the next pair's scores
issue while the pow still runs). The AV matmuls lag the exps by 6 slots
so PE's queue never head-of-line blocks; softmax normalization runs
straight off the AV psum (DVE reciprocal + Pool broadcast + DVE
multiply - the lag-6 window covers the bank reuse). LayerNorm stats
accumulate
in one [33, SQ] psum bank as each dt lands (Square/Identity on ACT),
FFN2 accumulation is interleaved per hsb pair with FFN1, and the hidden
activations carry a uniform x64 scale (relu bias via per-ft pointers,
with the DVE max-trick correction folded into b2). Post-attention
arithmetic is bf16 end to end.
"""

import numpy as np
import ml_dtypes

D = 512
S = 2048
B = 2
H = 8
HD = 64
F = 2048
EPS = 1e-5
NCORES = 8
SQ = 512          # queries per core
P = 128           # partitions
KD = D // P       # 4   D-tiles
KT = S // P       # 16  key tiles
TB = S // 512     # 4   512-token blocks
FT = F // P       # 16  FFN hidden tiles
HP = H // 2       # 4   head pairs
K2 = KT // 2      # 8   key-tile pairs
VC = 96           # padded AV columns: 64 values + 1 ones + 31 zeros

WS = 64.0         # host weight scale
SCALE_QKV = 1.0 / 16.0    # psum *  -> activations stored x4
SCALE_WO = 2.0 ** -12
SCALE_FF2 = 2.0 ** -8
ONES_COL = 1.0 / 16.0     # vh ones column value -> avt = 64*av
EXP_SCALE = 1.0 / 128.0   # scores psum = 16 * true score; softmax /8

BF16 = ml_dtypes.bfloat16
F8 = ml_dtypes.float8_e4m3

# k2 indices whose exp runs on DVE (tensor_tensor pow straight from PSUM;
# same DVE cost as the psum->sbuf copy the old gpsimd path needed), per side.
# Tuned so ACT (1038ns/exp) and DVE (1192ns/pow + fillers) stay balanced:
# hp0 carries v_proj on both engines, hp3 has PE-only wo fillers.
# slot 2 is kept pow-free on hp1+ - that's where both pav->sbuf normalize
# copies for the previous head pair burst onto DVE.
POOL_SETS = (
    ({0, 4, 7}, {2, 5}),         # slot-0/7 Pool slots bridge the head-pair
    ({0, 4, 7}, {2, 5}),         # boundaries: the DVE psum->sbuf copy
    ({0, 4, 7}, {2, 5}),         # releases the score bank early, so the
    ({0, 4, 7}, {2, 5}),         # next scores start while the pow runs
)
# FFN1 consume engines per ft PAIR: 'a' = ACT relu, 'd' = DVE max
# (gpsimd cannot read PSUM, so no Pool here)
FT_ENG = "adadadad"

_CACHE = {}
LAST_RESULT = None


def _build_nc():
    import concourse.bacc as bacc
    import concourse.tile as tile
    from concourse import mybir

    bf = mybir.dt.bfloat16
    f8 = mybir.dt.float8e4
    f32 = mybir.dt.float32
    ACT = mybir.ActivationFunctionType
    ALU = mybir.AluOpType
    DR = mybir.MatmulPerfMode.DoubleRow

    nc = bacc.Bacc("TRN2", target_bir_lowering=False, debug=False)

    def din(name, shape, dt=f8):
        return nc.dram_tensor(name, shape, dt, kind="ExternalInput").ap()

    t_q8 = din("q8", [P, KD, SQ])
    t_qbf = din("qbf16", [P, KD, SQ], bf)
    t_kt = din("kt8", [P, KD, S])
    t_vt = din("vt8", [P, KD, S])
    t_wq = din("wq8", [P, KD, D])
    t_wk = din("wk8", [P, KD, D])
    t_wv = din("wv8", [P, KD, D])
    t_wo = din("wo8", [HD, HP, 2, KD, P])
    t_w1 = din("w18", [P, KD, F])
    t_w2 = din("w28", [P, FT, D])
    t_bq = din("bq4", [P, KD], f32)
    t_bk = din("bk4", [P, KD], f32)
    t_b1m = din("b1m64", [P, FT], f32)
    t_b1p = din("b1p64", [P, FT], f32)
    t_g1 = din("g1", [P, KD], f32)
    t_be1 = din("be1", [P, KD], f32)
    t_be1p = din("be1p", [P, KD], f32)
    t_g2 = din("g2", [P, KD], f32)
    t_be2 = din("be2", [P, KD], f32)
    t_out = nc.dram_tensor("outT", [P, KD, SQ], bf, kind="ExternalOutput").ap()

    with tile.TileContext(nc) as tc, \
         tc.tile_pool(name="statics", bufs=1) as SP:
        def st(shape, dt, name):
            return SP.tile(shape, dt, tag=name, name=name)

        ones_bf = st([P, 1], bf, "ones_bf")
        warm_rhs = st([1, SQ], bf, "warm_rhs")
        ebase = st([P, 2, SQ], f32, "ebase")
        nhalf_t = st([1, SQ], f32, "nhalf_t")
        eps_w = st([1, 1], bf, "eps_w")
        ones_rhs = st([1, SQ], bf, "ones_rhs")
        sink0 = st([1, 1], f32, "sink0")
        sink1 = st([1, 1], f32, "sink1")

        # persistent activations
        q_z = st([P, KD, 2, SQ], f8, "q_z")         # slot1 = zeros
        kh = st([P, KD, S + P], f8, "kh")           # +128 zero pad for DR dup
        vh = st([P, KT, H, VC], f8, "vh")
        avt = st([P, H, SQ], f8, "avt")
        xres = st([P, KD, SQ], bf, "xres")          # residual; reused as r2
        x1b = st([P, KD, SQ], f8, "x1b")
        x1f = st([P, KD, SQ], bf, "x1f")
        hsb = st([P, FT, SQ], f8, "hsb")
        outsb = st([P, KD, SQ], bf, "outsb")

        nc.vector.memset(ones_bf, 1.0 / D)
        nc.vector.memset(warm_rhs, 0.0)
        nc.vector.memset(q_z[:, 0, 1, :], 0.0)
        nc.vector.memset(vh[:, 0:4, :, HD:VC], 0.0)

        # prime the Exp/Identity/Relu/Square act table at t=0 so the implicit
        # LoadActFuncSet never lands in front of real work
        nc.scalar.activation(out=sink0[0:1, 0:1], in_=ones_bf[0:1, 0:1],
                             func=ACT.Identity)

        # gpsimd memsets in need-by order (Pool runs them serially): exp
        # base, then only the hp0/k2=0-critical zero regions; everything
        # else is split so nothing big gates the first AV.
        nc.gpsimd.memset(ebase, float(np.exp(EXP_SCALE)))
        nc.gpsimd.memset(vh[:, 0:4, :, HD:HD + 1], ONES_COL)
        nc.gpsimd.memset(q_z[:, 1:4, 1, :], 0.0)
        nc.gpsimd.memset(vh[:, 4:10, :, HD:VC], 0.0)
        nc.gpsimd.memset(vh[:, 4:10, :, HD:HD + 1], ONES_COL)
        nc.gpsimd.memset(vh[:, 10:16, :, HD:VC], 0.0)
        nc.gpsimd.memset(vh[:, 10:16, :, HD:HD + 1], ONES_COL)
        nc.gpsimd.memset(kh[:, :, S:S + P], 0.0)
        nc.gpsimd.memset(nhalf_t, -0.5)
        nc.gpsimd.memset(eps_w, EPS)
        nc.gpsimd.memset(ones_rhs, 1.0)

        # ---- DMAs in need-by order; the SP sequencer issues one dma_start
        # every ~565ns, so the k2=0 critical set (wk, kt block0, bk, q8, wq,
        # bq) must be the first six issues.
        kt_sb = st([P, KD, S + P], f8, "kt_sb")     # K proj moving data
        vt_sb = st([P, KD, S], f8, "vt_sb")
        wk = st([P, KD, D], f8, "wk")
        nc.sync.dma_start(out=wk, in_=t_wk)
        nc.sync.dma_start(out=kt_sb[:, :, 0:512], in_=t_kt[:, :, 0:512])
        bk = st([P, KD], f32, "bk")
        nc.sync.dma_start(out=bk, in_=t_bk)
        q8 = st([P, KD, SQ], f8, "q8")
        nc.sync.dma_start(out=q8, in_=t_q8)
        wq = st([P, KD, D], f8, "wq")
        nc.sync.dma_start(out=wq, in_=t_wq)
        bq = st([P, KD], f32, "bq")
        nc.sync.dma_start(out=bq, in_=t_bq)
        wv = st([P, KD, D], f8, "wv")
        nc.sync.dma_start(out=wv, in_=t_wv)
        for tb in range(1, TB):
            tbs = slice(tb * 512, (tb + 1) * 512)
            nc.sync.dma_start(out=kt_sb[:, :, tbs], in_=t_kt[:, :, tbs])
            pbs = slice((tb - 1) * 512, tb * 512)
            nc.sync.dma_start(out=vt_sb[:, :, pbs], in_=t_vt[:, :, pbs])
        nc.sync.dma_start(out=vt_sb[:, :, 3 * 512:4 * 512],
                          in_=t_vt[:, :, 3 * 512:4 * 512])
        # tail-phase inputs, queued last
        qbf16 = st([P, KD, SQ], bf, "qbf16")
        nc.sync.dma_start(out=qbf16, in_=t_qbf)
        wo = st([HD, HP, 2, KD, P], f8, "wo")
        nc.sync.dma_start(out=wo, in_=t_wo)
        w1 = st([P, KD, F], f8, "w1")
        nc.sync.dma_start(out=w1, in_=t_w1)
        w2 = st([P, FT, D], f8, "w2")
        nc.sync.dma_start(out=w2, in_=t_w2)
        b1m = st([P, FT], f32, "b1m")
        nc.sync.dma_start(out=b1m, in_=t_b1m)
        b1p = st([P, FT], f32, "b1p")
        nc.sync.dma_start(out=b1p, in_=t_b1p)
        g1 = st([P, KD], f32, "g1")
        nc.sync.dma_start(out=g1, in_=t_g1)
        be1 = st([P, KD], f32, "be1")
        nc.sync.dma_start(out=be1, in_=t_be1)
        be1p = st([P, KD], f32, "be1p")
        nc.sync.dma_start(out=be1p, in_=t_be1p)
        g2 = st([P, KD], f32, "g2")
        nc.sync.dma_start(out=g2, in_=t_g2)
        be2 = st([P, KD], f32, "be2")
        nc.sync.dma_start(out=be2, in_=t_be2)
        SP.seal()

        # ============ phases 1+2: projections interleaved with attention
        with tc.tile_pool(name="att_sb", bufs=1) as asb, \
             tc.tile_pool(name="pj", bufs=2, space="PSUM") as pj, \
             tc.tile_pool(name="sc", bufs=1, space="PSUM") as scp, \
             tc.tile_pool(name="av", bufs=1, space="PSUM") as avp:

            def qgroup(dt, eng="d"):
                ps = pj.tile([P, SQ], f32, tag="pj", name=f"psq{dt}")
                for k in (0, 2):
                    nc.tensor.matmul(ps, wq[:, k:k + 2, dt * P:(dt + 1) * P],
                                     q8[:, k:k + 2, :], start=(k == 0),
                                     stop=(k == 2), perf_mode=DR)
                if eng == "a":
                    nc.scalar.activation(out=q_z[:, dt, 0, :], in_=ps,
                                         func=ACT.Identity, scale=SCALE_QKV,
                                         bias=bq[:, dt:dt + 1])
                else:
                    nc.vector.tensor_scalar(out=q_z[:, dt, 0, :], in0=ps,
                                            scalar1=SCALE_QKV,
                                            scalar2=bq[:, dt:dt + 1],
                                            op0=ALU.mult, op1=ALU.add)

            def kgroup(dt, tb, eng="d"):
                def go():
                    tbs = slice(tb * 512, (tb + 1) * 512)
                    ps = pj.tile([P, 512], f32, tag="pj", name=f"psk{dt}_{tb}")
                    for k in (0, 2):
                        nc.tensor.matmul(ps, wk[:, k:k + 2, dt * P:(dt + 1) * P],
                                         kt_sb[:, k:k + 2, tbs], start=(k == 0),
                                         stop=(k == 2), perf_mode=DR)
                    if eng == "a":
                        nc.scalar.activation(out=kh[:, dt, tbs], in_=ps,
                                             func=ACT.Identity, scale=SCALE_QKV,
                                             bias=bk[:, dt:dt + 1])
                    else:
                        nc.vector.tensor_scalar(out=kh[:, dt, tbs], in0=ps,
                                                scalar1=SCALE_QKV,
                                                scalar2=bk[:, dt:dt + 1],
                                                op0=ALU.mult, op1=ALU.add)
                return go

            def qk_groups(dt):
                # 1 ACT + 4 DVE consumers keeps both engines balanced
                return [lambda: qgroup(dt, "a")] + \
                    [kgroup(dt, tb, "d") for tb in range(TB)]

            def v_proj(tt, eng):
                ps = pj.tile([P, D], f32, tag="pj", name=f"psv{tt}")
                for k in (0, 2):
                    nc.tensor.matmul(ps, vt_sb[:, k:k + 2, tt * P:(tt + 1) * P],
                                     wv[:, k:k + 2, :], start=(k == 0),
                                     stop=(k == 2), perf_mode=DR)
                if eng == "a":
                    nc.scalar.activation(out=vh[:, tt, :, 0:HD],
                                         in_=ps.rearrange("p (h d) -> p h d", h=H),
                                         func=ACT.Copy, scale=SCALE_QKV)
                else:
                    nc.vector.tensor_scalar_mul(
                        vh[:, tt, :, 0:HD], ps.rearrange("p (h d) -> p h d", h=H),
                        SCALE_QKV)

            # Wo chains: po psum tiles live in the pj pool (projections are
            # done by the time these run).
            po_tiles = {}

            def wo_mm(dt, hp_i):
                def go():
                    if dt not in po_tiles:
                        po_tiles[dt] = pj.tile([P, SQ], f32, tag="pj",
                                               name=f"po{dt}")
                    nc.tensor.matmul(po_tiles[dt], wo[:, hp_i, :, dt, :],
                                     avt[0:HD, 2 * hp_i:2 * hp_i + 2, :],
                                     start=(hp_i == 0), stop=(hp_i == HP - 1),
                                     perf_mode=DR)
                return go

            def wo_consume(dt):
                po = po_tiles.pop(dt)
                nc.vector.scalar_tensor_tensor(
                    out=xres[:, dt, :], in0=po, scalar=SCALE_WO,
                    in1=qbf16[:, dt, :], op0=ALU.mult, op1=ALU.add)

            # keep-warm matmuls to cover the initial DMA wait
            warm_ps = pj.tile([P, SQ], f32, tag="pj", name="warm_ps")
            for w in range(5):
                nc.tensor.matmul(warm_ps[0:1, :], ones_bf[0:1, 0:1], warm_rhs,
                                 start=(w == 0), stop=(w == 4))
            nc.vector.tensor_scalar(out=sink0, in0=warm_ps[0:1, 0:1],
                                    scalar1=0.0, scalar2=0.0,
                                    op0=ALU.mult, op1=ALU.add)

            kgroup(0, 0, "d")()
            qgroup(0, "a")
            qgroup(1, "d")
            kgroup(1, 0, "a")()
            for tb in range(1, TB):
                kgroup(0, tb, "d")()
                kgroup(1, tb, "a")()
            fillers = []

            # ---- AV lags the exps by 2 slots so PE's in-order queue never
            # head-of-line blocks on an unfinished exp.  Normalize = one fast
            # DVE psum->sbuf copy (frees the pav bank), then Pool broadcasts
            # the denominator and divides - no DVE reciprocal/mul needed.
            pend = []       # (p_pair, k2, hp, (pav0, pav1)) awaiting AV
            norm_q = []     # per-side normalize closures, popped 1/slot

            def norm_side(hp_n, side, pv):
                h = 2 * hp_n + side
                rec = asb.tile([1, SQ], bf, tag=f"rec{side}", bufs=2,
                               name=f"rec{h}")
                # whole chain off psum (no evacuation copy); AV lag-3 gives
                # the next head pair enough slack before the bank frees at
                # the mul.  Last head pair's mul on DVE - it gates Wo + LN1.
                with nc.allow_low_precision(reason="softmax, bf16 ok"):
                    nc.vector.reciprocal(rec, pv[HD:HD + 1, :])
                rbc = asb.tile([HD, SQ], bf, tag=f"rbc{side}", bufs=2,
                               name=f"rbc{h}")
                nc.gpsimd.partition_broadcast(rbc, rec)
                with nc.allow_low_precision(reason="softmax, f8 out"):
                    nc.vector.tensor_mul(avt[0:HD, h, :], pv[0:HD, :], rbc)

            def emit_av():
                pp, pk2, php, pav = pend.pop(0)
                for side in (0, 1):
                    nc.tensor.matmul(
                        pav[side][0:VC, :],
                        vh[:, 2 * pk2:2 * pk2 + 2, 2 * php + side, :],
                        pp[side], start=(pk2 == 0), stop=(pk2 == K2 - 1),
                        perf_mode=DR)
                if pk2 == K2 - 1:
                    norm_q.append(lambda s=0, pv=pav[0]: norm_side(php, s, pv))
                    norm_q.append(lambda s=1, pv=pav[1]: norm_side(php, s, pv))

            for hp in range(HP):
                if hp == 1:
                    fillers += qk_groups(2)
                elif hp == 2:
                    fillers += qk_groups(3)
                elif hp == 3:
                    fillers += [wo_mm(dt, i) for i in range(2) for dt in (0, 1)]
                pav = (avp.tile([P, SQ], f32, tag="av0", name=f"pav0_{hp}"),
                       avp.tile([P, SQ], f32, tag="av1", name=f"pav1_{hp}"))
                for k2 in range(K2):
                    pscs = []
                    for side in range(2):
                        psc = scp.tile([P, 2, SQ], f32, tag=f"sc{side}", bufs=1)
                        pr = slice(side * HD, side * HD + HD)
                        for i in range(2):
                            kt = 2 * k2 + i
                            lw = kh[pr, hp, kt * P:(kt + 2) * P].rearrange(
                                "p (two k) -> p two k", two=2)
                            nc.tensor.matmul(psc[:, i, :], lw,
                                             q_z[pr, hp, :, :], start=True,
                                             stop=True, perf_mode=DR)
                        pscs.append(psc)
                    for side in range(2):
                        p = asb.tile([P, 2, SQ], f8, tag=f"p{side}", bufs=7)
                        if k2 in POOL_SETS[hp][side]:
                            scb = asb.tile([P, 2, SQ], bf, tag=f"scb{side}",
                                           bufs=2)
                            nc.vector.tensor_copy(scb, pscs[side])
                            with nc.allow_low_precision(reason="softmax f8"):
                                nc.gpsimd.tensor_tensor(p, ebase, scb,
                                                        ALU.pow)
                        else:
                            nc.scalar.activation(out=p, in_=pscs[side],
                                                 func=ACT.Exp, scale=EXP_SCALE)
                        pscs[side] = p
                    # both sides' normalize bursts in one slot so the k2==0
                    # AV below never sits in the PE queue waiting on a pcp
                    # copy that hasn't been emitted yet
                    while norm_q:
                        norm_q.pop(0)()
                    if len(pend) >= 6:
                        emit_av()
                    if hp == 0:
                        v_proj(2 * k2, "a")
                        v_proj(2 * k2 + 1, "d")
                    elif fillers:
                        fillers.pop(0)()
                    pend.append((pscs, k2, hp, pav))

            while pend:
                emit_av()
            while fillers:
                fillers.pop(0)()
            for dt in (0, 1):
                wo_mm(dt, 2)()
            while norm_q:
                norm_q.pop(0)()

            # finish Wo for dt 0/1 (head pair 3) + residual
            for dt in (0, 1):
                wo_mm(dt, 3)()
                wo_consume(dt)

        # ============ phase 3: Wo tail + LN1 ============
        # LN helper pieces.  Stats accumulate into psum as each src dt lands;
        # the scalar chain then runs on DVE (rstd via tensor_scalar pow, so
        # Pool only does the two broadcasts); per-dt transforms keep bf16
        # SBUF operands so DVE's 2x/4x fast modes kick in, with the f8 x1b
        # store handled by Pool.
        def ln_stats(stp, src_dt, sq_pool, first, last, eng="d"):
            sq = sq_pool.tile([P, SQ], bf, tag="sq", bufs=2)
            if eng == "a":
                nc.scalar.activation(out=sq, in_=src_dt, func=ACT.Square)
            else:
                nc.vector.tensor_mul(sq, src_dt, src_dt)
            nc.tensor.matmul(ln_ps[0:1, :], ones_bf, src_dt, start=first,
                             stop=last)
            nc.tensor.matmul(ln_ps[32:33, :], ones_bf, sq, start=False,
                             stop=last)

        def ln_chain(tmp, nwarm, wtile):
            if nwarm:
                for w in range(nwarm):
                    nc.tensor.matmul(wtile[0:1, :], ones_bf[0:1, 0:1],
                                     warm_rhs, start=(w == 0),
                                     stop=(w == nwarm - 1))
            m2 = tmp.tile([1, SQ], f32, tag="ln_m2")
            nc.scalar.activation(out=m2, in_=ln_ps[0:1, :], func=ACT.Square)
            var = tmp.tile([1, SQ], f32, tag="ln_var")
            nc.vector.scalar_tensor_tensor(out=var, in0=m2, scalar=-1.0,
                                           in1=ln_ps[32:33, :], op0=ALU.mult,
                                           op1=ALU.add)
            rstd = tmp.tile([1, SQ], bf, tag="ln_rstd")
            with nc.allow_low_precision(reason="LN rstd, bf16 ok"):
                nc.gpsimd.tensor_tensor(rstd, var, nhalf_t, ALU.pow)
            cvec = tmp.tile([1, SQ], bf, tag="ln_c")
            with nc.allow_low_precision(reason="LN shift, bf16 ok"):
                nc.vector.scalar_tensor_tensor(out=cvec, in0=ln_ps[0:1, :],
                                               scalar=1.0, in1=rstd,
                                               op0=ALU.mult, op1=ALU.mult)
            pA = tmp.tile([P, SQ], bf, tag="bA")
            nc.gpsimd.partition_broadcast(pA, rstd)
            pC = tmp.tile([P, SQ], bf, tag="bC")
            nc.gpsimd.partition_broadcast(pC, cvec)
            return pA, pC

        with tc.tile_pool(name="ln1_sb", bufs=1) as tmp1, \
             tc.tile_pool(name="po2", bufs=2, space="PSUM") as pop, \
             tc.tile_pool(name="st1", bufs=1, space="PSUM") as stp1, \
             tc.tile_pool(name="wm1", bufs=1, space="PSUM") as wmp1:
            ln_ps = stp1.tile([33, SQ], f32, tag="s12")
            # seed the sq row with eps so var+eps falls out of the sum chain
            nc.tensor.matmul(ln_ps[32:33, :], eps_w, ones_rhs, start=True,
                             stop=False)
            for dt in (0, 1):
                ln_stats(stp1, xres[:, dt, :], tmp1, dt == 0, False, "a")
            for dt in (2, 3):
                po_tiles[dt] = pop.tile([P, SQ], f32, tag="po", name=f"po{dt}")
                for hp_i in range(HP):
                    nc.tensor.matmul(po_tiles[dt], wo[:, hp_i, :, dt, :],
                                     avt[0:HD, 2 * hp_i:2 * hp_i + 2, :],
                                     start=(hp_i == 0), stop=(hp_i == HP - 1),
                                     perf_mode=DR)
                wo_consume(dt)
                ln_stats(stp1, xres[:, dt, :], tmp1, False, dt == 3, "a")
            wm1 = wmp1.tile([P, SQ], f32, tag="wm", name="lnwarm")
            pA1, pC1 = ln_chain(tmp1, 4, wm1)
            for dt in range(KD):
                t1 = tmp1.tile([P, SQ], bf, tag="t1", bufs=2)
                nc.vector.tensor_mul(t1, xres[:, dt, :], pA1)
                nc.vector.tensor_sub(t1, t1, pC1)
                if dt < 2:
                    nc.scalar.activation(out=x1b[:, dt, :], in_=t1,
                                         func=ACT.Identity,
                                         scale=g1[:, dt:dt + 1],
                                         bias=be1[:, dt:dt + 1])
                else:
                    nc.vector.tensor_scalar(out=x1b[:, dt, :], in0=t1,
                                            scalar1=g1[:, dt:dt + 1],
                                            scalar2=be1[:, dt:dt + 1],
                                            op0=ALU.mult, op1=ALU.add)
                nc.vector.tensor_scalar(out=x1f[:, dt, :], in0=t1,
                                        scalar1=g1[:, dt:dt + 1],
                                        scalar2=be1p[:, dt:dt + 1],
                                        op0=ALU.mult, op1=ALU.add)
            nc.vector.tensor_scalar(out=sink1, in0=wm1[0:1, 0:1],
                                    scalar1=0.0, scalar2=0.0,
                                    op0=ALU.mult, op1=ALU.add)

        # ============ phase 4: FFN + LN2 stats interleaved ============
        # FFN2 accumulation runs per hsb pair as FFN1 produces it - no
        # barrier between the two GEMMs - and each FFN2 consumer feeds the
        # LN2 stat matmuls immediately.
        with tc.tile_pool(name="ffn_sb", bufs=1) as tmp2, \
             tc.tile_pool(name="pf", bufs=3, space="PSUM") as pfp, \
             tc.tile_pool(name="py", bufs=1, space="PSUM") as pyp, \
             tc.tile_pool(name="st2", bufs=1, space="PSUM") as stp2:
            ln_ps = stp2.tile([33, SQ], f32, tag="s12")
            nc.tensor.matmul(ln_ps[32:33, :], eps_w, ones_rhs, start=True,
                             stop=False)
            py_t = [pyp.tile([P, SQ], f32, tag=f"py{dt}", name=f"py{dt}")
                    for dt in range(KD)]
            for ft in range(FT):
                pf = pfp.tile([P, SQ], f32, tag="pf")
                nc.tensor.matmul(pf, w1[:, 0:2, ft * P:(ft + 1) * P],
                                 x1b[:, 0:2, :], start=True, stop=False,
                                 perf_mode=DR)
                nc.tensor.matmul(pf, w1[:, 2:4, ft * P:(ft + 1) * P],
                                 x1b[:, 2:4, :], start=False, stop=True,
                                 perf_mode=DR)
                if ft % 2 == 0:
                    nc.scalar.activation(out=hsb[:, ft, :], in_=pf,
                                         func=ACT.Relu,
                                         bias=b1p[:, ft:ft + 1])
                else:
                    # max(64h, -64b1) = 64(relu(h+b1) - b1); the -b1 term is
                    # folded into b2 on the host for these columns
                    nc.vector.tensor_scalar(out=hsb[:, ft, :], in0=pf,
                                            scalar1=b1m[:, ft:ft + 1],
                                            scalar2=None, op0=ALU.max)
                if ft % 2 == 1:
                    f = ft - 1
                    for dt in range(KD):
                        nc.tensor.matmul(py_t[dt],
                                         w2[:, f:f + 2, dt * P:(dt + 1) * P],
                                         hsb[:, f:f + 2, :], start=(f == 0),
                                         stop=(f == FT - 2), perf_mode=DR)
            for dt in range(KD):
                nc.vector.scalar_tensor_tensor(
                    out=xres[:, dt, :], in0=py_t[dt], scalar=SCALE_FF2,
                    in1=x1f[:, dt, :], op0=ALU.mult, op1=ALU.add)
                ln_stats(stp2, xres[:, dt, :], tmp2, dt == 0, dt == 3, "a")

            # ======== LN2 + output (stats already accumulated) ========
            pA2, pC2 = ln_chain(tmp2, 0, None)
            for dt in range(KD):
                t1 = tmp2.tile([P, SQ], bf, tag="t1", bufs=2)
                nc.vector.tensor_mul(t1, xres[:, dt, :], pA2)
                nc.vector.tensor_sub(t1, t1, pC2)
                if dt % 2 == 0:
                    nc.vector.tensor_scalar(out=outsb[:, dt, :], in0=t1,
                                            scalar1=g2[:, dt:dt + 1],
                                            scalar2=be2[:, dt:dt + 1],
                                            op0=ALU.mult, op1=ALU.add)
                else:
                    nc.scalar.activation(out=outsb[:, dt, :], in_=t1,
                                         func=ACT.Identity,
                                         scale=g2[:, dt:dt + 1],
                                         bias=be2[:, dt:dt + 1])
                nc.sync.dma_start(out=t_out[:, dt, :], in_=outsb[:, dt, :])

    nc.compile()
    return nc


def _get_nc():
    if "nc" not in _CACHE:
        _CACHE["nc"] = _build_nc()
    return _CACHE["nc"]


def make_in_maps(q, k, v, Wq, bq, Wk, bk, Wv, bv, Wo, bo, W1, b1, W2, b2,
                 g1, be1, g2, be2):
    f32 = np.float32
    q = np.asarray(q, f32)
    k = np.asarray(k, f32)
    v = np.asarray(v, f32)
    Wq, Wk, Wv, Wo = (np.asarray(x, f32) for x in (Wq, Wk, Wv, Wo))
    W1, W2 = np.asarray(W1, f32), np.asarray(W2, f32)
    bq, bk, bv, bo = (np.asarray(x, f32) for x in (bq, bk, bv, bo))
    b1, b2 = np.asarray(b1, f32), np.asarray(b2, f32)
    g1, be1, g2, be2 = (np.asarray(x, f32) for x in (g1, be1, g2, be2))

    def tile_pd(x, n):  # [n*P] -> [P, n]
        return np.ascontiguousarray(x.reshape(n, P).T)

    def wt8(w, cols):  # [out, in] -> [P, in//P, cols] scaled fp8
        return np.ascontiguousarray(
            (w.T * WS).reshape(-1, P, cols).transpose(1, 0, 2)).astype(F8)

    # hsb rows hold 64*relu(h) uniformly -> W2 columns just need x4
    w28 = np.ascontiguousarray(
        (W2.T * 4.0).reshape(FT, P, D).transpose(1, 0, 2)).astype(F8)

    dve_mask = np.zeros(F, bool)
    for ft in range(1, FT, 2):
        dve_mask[ft * P:(ft + 1) * P] = True
    bo_eff = bo + Wo @ bv
    b2_eff = b2 + W2[:, dve_mask] @ b1[dve_mask]

    wo8 = np.ascontiguousarray(
        (Wo.T * WS).reshape(HP, 2, HD, KD, P).transpose(2, 0, 1, 3, 4)).astype(F8)

    shared = {
        "wq8": wt8(Wq, D), "wk8": wt8(Wk, D), "wv8": wt8(Wv, D),
        "w18": wt8(W1, F), "w28": w28, "wo8": wo8,
        "bq4": tile_pd(4.0 * bq, KD), "bk4": tile_pd(4.0 * bk, KD),
        "b1m64": tile_pd(-64.0 * b1, FT), "b1p64": tile_pd(64.0 * b1, FT),
        "g1": tile_pd(g1, KD), "be1": tile_pd(be1, KD),
        "be1p": tile_pd(be1 + b2_eff, KD),
        "g2": tile_pd(g2, KD), "be2": tile_pd(be2, KD),
    }

    def fm(x, dt):  # [S, D] -> [P, KD, S] feature-major
        return np.ascontiguousarray(
            x.T.reshape(KD, P, -1).transpose(1, 0, 2)).astype(dt)

    kts = [fm(k[b], F8) for b in range(B)]
    vts = [fm(v[b], F8) for b in range(B)]

    in_maps = []
    for c in range(NCORES):
        b, s0 = c // 4, (c % 4) * SQ
        qs = q[b, s0:s0 + SQ, :]
        in_maps.append({
            "q8": fm(qs, F8),
            "qbf16": fm(qs + bo_eff, BF16),
            "kt8": kts[b], "vt8": vts[b], **shared,
        })
    return in_maps


def assemble_out(results):
    out = np.empty((B, S, D), np.float32)
    for c in range(NCORES):
        b, s0 = c // 4, (c % 4) * SQ
        out[b, s0:s0 + SQ, :] = results[c]["outT"].astype(np.float32) \
            .transpose(2, 1, 0).reshape(SQ, D)
    return out


def kernel(**inputs):
    global LAST_RESULT
    import os

    from concourse.bass_utils import run_bass_kernel_spmd

    nc = _get_nc()
    in_maps = make_in_maps(**inputs)
    try:
        res = run_bass_kernel_spmd(nc, in_maps, core_ids=list(range(NCORES)))
    except ModuleNotFoundError:
        # BASS_TRACE set but this container has no axon NTFF profile hook
        # (antenv.axon_hooks missing) — rerun untraced.
        os.environ["BASS_NEVER_TRACE"] = "1"
        res = run_bass_kernel_spmd(nc, in_maps, core_ids=list(range(NCORES)))
    LAST_RESULT = res
    return assemble_out(res.results)

